# revision 18
# baseline (speedup 1.0000x reference)
"""Attention kernel for Trainium2 (Bass/Tile), 8-core SPMD.

Problem: x[32,1024,768]; Q/K/V = x @ W.T + b (768->768); S = Q K^T / sqrt(768);
P = softmax(S, axis=-1); out = P V.

Sharding: pure data-parallel over batch — 4 batches per core, no collectives.
Host-side prep: x is passed per-batch transposed (xT [d, n]) and the weights
pre-transposed (W^T [d, o]) so every matmul operand already has its
contraction dim on partitions.

Per-core dataflow (per batch), all matmul operands fp16 (full PE rate, FWL
weight loads), fp32 PSUM accumulation:
  - Qt/Kt [o, n]: lhsT = W^T tile, rhs = xT; bias fused into the
    PSUM->SBUF copy on ACT (per-partition bias)
  - V [n, o] natural: lhsT = xT tile, rhs = Wv^T; bias via DVE add with a
    partition-broadcast bias tile; stored fp16 with an appended ones column
  - S^T per k-chunk: lhsT = Kt slice, rhs = Qt; exp(S^T * 1/c) on ACT
    straight out of PSUM, written fp16. No max-subtraction: logits are
    ~N(0,1), |logit| < ~9, so exp() stays finite in fp32/fp16 and the
    result is mathematically identical to the max-subtracted softmax.
  - PV per q-chunk: lhsT = exp(S^T) slice, rhs = V_aug; the ones column
    of V_aug yields the softmax row-sums in the output's last column
  - final PSUM->SBUF copy on ACT applies the 1/rowsum normalization
"""

import math

import numpy as np

import concourse.bass as bass
import concourse.mybir as mybir
import concourse.tile as tile
from concourse import bacc
from concourse.bass_utils import run_bass_kernel_spmd

F32 = mybir.dt.float32
F32R = mybir.dt.float32r
F16 = mybir.dt.float16
BF16 = mybir.dt.bfloat16

N_CORES = 8
B_TOTAL = 32
B = B_TOTAL // N_CORES  # batches per core
N = 1024  # sequence length
D = 768  # embed dim
O = 768  # out dim
P = 128  # partitions
ND = D // P  # 6 d-chunks
NO = O // P  # 6 o-chunks
NQ = N // P  # 8 seq chunks
OA = O + 1  # V width incl. the ones column
SCALE = math.sqrt(float(O)) + 1e-6
INV_C = float(1.0 / SCALE)

Act = mybir.ActivationFunctionType


def build():
    nc = bacc.Bacc("TRN2", target_bir_lowering=False, debug=False)

    xT_d = nc.dram_tensor("xT", [B, D, N], F16, kind="ExternalInput").ap()
    wq_d = nc.dram_tensor("wqT", [D, O], F16, kind="ExternalInput").ap()
    wk_d = nc.dram_tensor("wkT", [D, O], F16, kind="ExternalInput").ap()
    wv_d = nc.dram_tensor("wvT", [D, O], F16, kind="ExternalInput").ap()
    bq_d = nc.dram_tensor("bq", [O], F32, kind="ExternalInput").ap()
    bk_d = nc.dram_tensor("bk", [O], F32, kind="ExternalInput").ap()
    bv_d = nc.dram_tensor("bv", [O], F32, kind="ExternalInput").ap()
    out_d = nc.dram_tensor("out", [B, N, D], F32, kind="ExternalOutput").ap()

    with tile.TileContext(nc) as tc:
        with (
            tc.tile_pool(name="const", bufs=1) as const_pool,
            tc.tile_pool(name="big", bufs=1) as big_pool,
            tc.tile_pool(name="xTp", bufs=2) as xT_pool,
            tc.tile_pool(name="small", bufs=2) as small_pool,
            tc.tile_pool(name="on", bufs=4) as on_pool,
            tc.tile_pool(name="ps", bufs=4, space="PSUM") as ps_pool,
        ):
            def load_xT(b):
                t = xT_pool.tile([P, ND, N], F16, tag="xT", name=f"xT{b}")
                for dd in range(ND):
                    nc.sync.dma_start(t[:, dd, :], xT_d[b, dd * P : (dd + 1) * P, :])
                return t

            # Startup loads: the Sync engine issues DMA descriptors at only
            # ~0.7us each, so spread issue across idle engines and put the
            # first-needed tensors (xT batch 0, wq) on the fastest path.
            w_sb = {nm: const_pool.tile([P, ND, O], F16, tag=f"w{nm}", name=f"w{nm}")
                    for nm in ("q", "k", "v")}
            xT_next = xT_pool.tile([P, ND, N], F16, tag="xT", name="xT0")
            for dd in range(ND):
                nc.sync.dma_start(xT_next[:, dd, :], xT_d[0, dd * P : (dd + 1) * P, :])
                (nc.gpsimd if dd % 2 == 0 else nc.scalar).dma_start(
                    w_sb["q"][:, dd, :], wq_d[dd * P : (dd + 1) * P, :]
                )
            for nm, wd in (("k", wk_d), ("v", wv_d)):
                for dd in range(ND):
                    (nc.scalar if dd % 2 == 0 else nc.gpsimd).dma_start(
                        w_sb[nm][:, dd, :], wd[dd * P : (dd + 1) * P, :]
                    )
            # per-partition bias layout for Qt/Kt ([o-chunk] -> [128,1] slices)
            bq_sb = const_pool.tile([P, NO], F32, tag="bq")
            nc.sync.dma_start(
                bq_sb, bass.AP(tensor=bq_d.tensor, offset=bq_d.offset, ap=[[1, P], [P, NO]])
            )
            bk_sb = const_pool.tile([P, NO], F32, tag="bk")
            nc.sync.dma_start(
                bk_sb, bass.AP(tensor=bk_d.tensor, offset=bk_d.offset, ap=[[1, P], [P, NO]])
            )
            # bv broadcast across partitions for the V add
            bvb = const_pool.tile([P, O], F32, tag="bvb")
            nc.gpsimd.dma_start(
                bvb, bass.AP(tensor=bv_d.tensor, offset=bv_d.offset, ap=[[0, P], [1, O]])
            )

            for b in range(B):
                xT = xT_next

                # ---- projections ----
                qT = big_pool.tile([P, NO, N], F16, tag="qT")
                kT = big_pool.tile([P, NO, N], F16, tag="kT")
                for nm, dst, bias in (("q", qT, bq_sb), ("k", kT, bk_sb)):
                    w = w_sb[nm]
                    for j in range(NO):
                        pp = ps_pool.tile([P, N], F32, tag="ps")
                        for dd in range(ND):
                            lw = w[:, dd, j * P : (j + 1) * P]
                            for h in range(2):
                                nc.tensor.matmul(
                                    pp[:, h * 512 : (h + 1) * 512],
                                    lw,
                                    xT[:, dd, h * 512 : (h + 1) * 512],
                                    start=(dd == 0),
                                    stop=(dd == ND - 1),
                                )
                        nc.scalar.activation(
                            dst[:, j, :], pp, Act.Identity, bias=bias[:, j : j + 1]
                        )

                v_sb = big_pool.tile([P, NQ, OA], F16, tag="v")
                nc.gpsimd.memset(v_sb[:, :, O : O + 1], 1.0)
                wv = w_sb["v"]
                for i in range(NQ):
                    pp = ps_pool.tile([P, O], F32, tag="ps")
                    for dd in range(ND):
                        lx = xT[:, dd, i * P : (i + 1) * P]
                        nc.tensor.matmul(
                            pp[:, 0:512], lx, wv[:, dd, 0:512],
                            start=(dd == 0), stop=(dd == ND - 1),
                        )
                        nc.tensor.matmul(
                            pp[:, 512:O], lx, wv[:, dd, 512:O],
                            start=(dd == 0), stop=(dd == ND - 1),
                        )
                    nc.vector.tensor_add(v_sb[:, i, 0:O], pp, bvb)

                # prefetch next batch's activations while attention runs
                if b + 1 < B:
                    xT_next = load_xT(b + 1)

                # ---- S^T + exp ----
                eT = big_pool.tile([P, NQ, N], F16, tag="eT")
                for kk in range(NQ):
                    sp = ps_pool.tile([P, N], F32, tag="ps")
                    for oo in range(NO):
                        lk = kT[:, oo, kk * P : (kk + 1) * P]
                        for h in range(2):
                            nc.tensor.matmul(
                                sp[:, h * 512 : (h + 1) * 512],
                                lk,
                                qT[:, oo, h * 512 : (h + 1) * 512],
                                start=(oo == 0),
                                stop=(oo == NO - 1),
                            )
                    nc.scalar.activation(
                        eT[:, kk, :], sp, Act.Exp, bias=0.0, scale=INV_C
                    )

                # ---- PV + normalize ----
                for i in range(NQ):
                    op_ = ps_pool.tile([P, OA], F32, tag="ps")
                    for kk in range(NQ):
                        le = eT[:, kk, i * P : (i + 1) * P]
                        nc.tensor.matmul(
                            op_[:, 0:512], le, v_sb[:, kk, 0:512],
                            start=(kk == 0), stop=(kk == NQ - 1),
                        )
                        nc.tensor.matmul(
                            op_[:, 512:OA], le, v_sb[:, kk, 512:OA],
                            start=(kk == 0), stop=(kk == NQ - 1),
                        )
                    rs = small_pool.tile([P, 1], F32, tag="rs")
                    nc.vector.reciprocal(rs, op_[:, O : O + 1])
                    on = on_pool.tile([P, O], F32, tag="on")
                    nc.scalar.activation(on, op_[:, 0:O], Act.Copy, bias=0.0, scale=rs)
                    # spread stores across both idle DMA issuers; a single
                    # HWDGE queue drains ~39GB/s and backlogs at the tail
                    e0, e1 = (nc.sync, nc.scalar) if i % 2 == 0 else (nc.scalar, nc.sync)
                    e0.dma_start(out_d[b, i * P : (i + 1) * P, 0:384], on[:, 0:384])
                    e1.dma_start(out_d[b, i * P : (i + 1) * P, 384:O], on[:, 384:O])

    nc.compile()
    return nc


_NC = None


def _get_nc():
    global _NC
    if _NC is None:
        _NC = build()
    return _NC


def run(inputs, trace=False):
    x = np.asarray(inputs["x"], dtype=np.float32)
    wqT = np.ascontiguousarray(np.asarray(inputs["Wq"], dtype=np.float32).T.astype(np.float16))
    wkT = np.ascontiguousarray(np.asarray(inputs["Wk"], dtype=np.float32).T.astype(np.float16))
    wvT = np.ascontiguousarray(np.asarray(inputs["Wv"], dtype=np.float32).T.astype(np.float16))
    bq = np.asarray(inputs["bq"], dtype=np.float32)
    bk = np.asarray(inputs["bk"], dtype=np.float32)
    bv = np.asarray(inputs["bv"], dtype=np.float32)

    xT = np.ascontiguousarray(x.transpose(0, 2, 1).astype(np.float16))  # [32, D, N]

    nc = _get_nc()
    in_maps = []
    for c in range(N_CORES):
        in_maps.append(
            {
                "xT": np.ascontiguousarray(xT[c * B : (c + 1) * B]),
                "wqT": wqT, "wkT": wkT, "wvT": wvT,
                "bq": bq, "bk": bk, "bv": bv,
            }
        )
    res = run_bass_kernel_spmd(
        nc, in_maps, core_ids=list(range(N_CORES)), trace=trace
    )
    out = np.concatenate([res.results[c]["out"] for c in range(N_CORES)], axis=0)
    return out, res


def kernel(**inputs):
    import os

    # tracing needs an NTFF hook that may be absent in the runtime env
    os.environ["BASS_NEVER_TRACE"] = "1"
    out, _ = run(inputs, trace=False)
    return out


# revision 19
# speedup vs baseline: 1.0073x; 1.0073x over previous
"""Attention kernel for Trainium2 (Bass/Tile), 8-core SPMD.

Problem: x[32,1024,768]; Q/K/V = x @ W.T + b (768->768); S = Q K^T / sqrt(768);
P = softmax(S, axis=-1); out = P V.

Sharding: pure data-parallel over batch — 4 batches per core, no collectives.
Host-side prep: x is passed per-batch transposed (xT [d, n]) and the weights
pre-transposed (W^T [d, o]) so every matmul operand already has its
contraction dim on partitions.

Per-core dataflow (per batch), all matmul operands fp16 (full PE rate, FWL
weight loads), fp32 PSUM accumulation:
  - Qt/Kt [o, n]: lhsT = W^T tile, rhs = xT; bias fused into the
    PSUM->SBUF copy on ACT (per-partition bias)
  - V [n, o] natural: lhsT = xT tile, rhs = Wv^T; bias via DVE add with a
    partition-broadcast bias tile; stored fp16 with an appended ones column
  - S^T per k-chunk: lhsT = Kt slice, rhs = Qt; exp(S^T * 1/c) on ACT
    straight out of PSUM, written fp16. No max-subtraction: logits are
    ~N(0,1), |logit| < ~9, so exp() stays finite in fp32/fp16 and the
    result is mathematically identical to the max-subtracted softmax.
  - PV per q-chunk: lhsT = exp(S^T) slice, rhs = V_aug; the ones column
    of V_aug yields the softmax row-sums in the output's last column
  - final PSUM->SBUF copy on ACT applies the 1/rowsum normalization
"""

import math

import numpy as np

import concourse.bass as bass
import concourse.mybir as mybir
import concourse.tile as tile
from concourse import bacc
from concourse.bass_utils import run_bass_kernel_spmd

F32 = mybir.dt.float32
F32R = mybir.dt.float32r
F16 = mybir.dt.float16
BF16 = mybir.dt.bfloat16

N_CORES = 8
B_TOTAL = 32
B = B_TOTAL // N_CORES  # batches per core
N = 1024  # sequence length
D = 768  # embed dim
O = 768  # out dim
P = 128  # partitions
ND = D // P  # 6 d-chunks
NO = O // P  # 6 o-chunks
NQ = N // P  # 8 seq chunks
OA = O + 1  # V width incl. the ones column
SCALE = math.sqrt(float(O)) + 1e-6
INV_C = float(1.0 / SCALE)

Act = mybir.ActivationFunctionType


def build():
    nc = bacc.Bacc("TRN2", target_bir_lowering=False, debug=False)

    xT_d = nc.dram_tensor("xT", [B, D, N], F16, kind="ExternalInput").ap()
    wq_d = nc.dram_tensor("wqT", [D, O], F16, kind="ExternalInput").ap()
    wk_d = nc.dram_tensor("wkT", [D, O], F16, kind="ExternalInput").ap()
    wv_d = nc.dram_tensor("wvT", [D, O], F16, kind="ExternalInput").ap()
    bq_d = nc.dram_tensor("bq", [O], F32, kind="ExternalInput").ap()
    bk_d = nc.dram_tensor("bk", [O], F32, kind="ExternalInput").ap()
    bv_d = nc.dram_tensor("bv", [O], F32, kind="ExternalInput").ap()
    out_d = nc.dram_tensor("out", [B, N, D], F32, kind="ExternalOutput").ap()

    with tile.TileContext(nc) as tc:
        with (
            tc.tile_pool(name="const", bufs=1) as const_pool,
            tc.tile_pool(name="big", bufs=1) as big_pool,
            tc.tile_pool(name="xTp", bufs=2) as xT_pool,
            tc.tile_pool(name="small", bufs=2) as small_pool,
            tc.tile_pool(name="on", bufs=4) as on_pool,
            tc.tile_pool(name="ps", bufs=4, space="PSUM") as ps_pool,
        ):
            def load_xT(b):
                t = xT_pool.tile([P, ND, N], F16, tag="xT", name=f"xT{b}")
                for dd in range(ND):
                    nc.sync.dma_start(t[:, dd, :], xT_d[b, dd * P : (dd + 1) * P, :])
                return t

            # Startup loads: the Sync engine issues DMA descriptors at only
            # ~0.7us each, so spread issue across idle engines and put the
            # first-needed tensors (xT batch 0, wq) on the fastest path.
            w_sb = {nm: const_pool.tile([P, ND, O], F16, tag=f"w{nm}", name=f"w{nm}")
                    for nm in ("q", "k", "v")}
            xT_next = xT_pool.tile([P, ND, N], F16, tag="xT", name="xT0")
            for dd in range(ND):
                nc.sync.dma_start(xT_next[:, dd, :], xT_d[0, dd * P : (dd + 1) * P, :])
                nc.gpsimd.dma_start(w_sb["q"][:, dd, :], wq_d[dd * P : (dd + 1) * P, :])
            for nm, wd in (("k", wk_d), ("v", wv_d)):
                for dd in range(ND):
                    nc.scalar.dma_start(w_sb[nm][:, dd, :], wd[dd * P : (dd + 1) * P, :])
            # per-partition bias layout for Qt/Kt ([o-chunk] -> [128,1] slices)
            bq_sb = const_pool.tile([P, NO], F32, tag="bq")
            nc.sync.dma_start(
                bq_sb, bass.AP(tensor=bq_d.tensor, offset=bq_d.offset, ap=[[1, P], [P, NO]])
            )
            bk_sb = const_pool.tile([P, NO], F32, tag="bk")
            nc.sync.dma_start(
                bk_sb, bass.AP(tensor=bk_d.tensor, offset=bk_d.offset, ap=[[1, P], [P, NO]])
            )
            # bv broadcast across partitions for the V add
            bvb = const_pool.tile([P, O], F32, tag="bvb")
            nc.gpsimd.dma_start(
                bvb, bass.AP(tensor=bv_d.tensor, offset=bv_d.offset, ap=[[0, P], [1, O]])
            )

            for b in range(B):
                xT = xT_next

                # ---- projections ----
                qT = big_pool.tile([P, NO, N], F16, tag="qT")
                kT = big_pool.tile([P, NO, N], F16, tag="kT")
                for nm, dst, bias in (("q", qT, bq_sb), ("k", kT, bk_sb)):
                    w = w_sb[nm]
                    for j in range(NO):
                        pp = ps_pool.tile([P, N], F32, tag="ps")
                        for dd in range(ND):
                            lw = w[:, dd, j * P : (j + 1) * P]
                            for h in range(2):
                                nc.tensor.matmul(
                                    pp[:, h * 512 : (h + 1) * 512],
                                    lw,
                                    xT[:, dd, h * 512 : (h + 1) * 512],
                                    start=(dd == 0),
                                    stop=(dd == ND - 1),
                                )
                        nc.scalar.activation(
                            dst[:, j, :], pp, Act.Identity, bias=bias[:, j : j + 1]
                        )

                v_sb = big_pool.tile([P, NQ, OA], F16, tag="v")
                nc.gpsimd.memset(v_sb[:, :, O : O + 1], 1.0)
                wv = w_sb["v"]
                for i in range(NQ):
                    pp = ps_pool.tile([P, O], F32, tag="ps")
                    for dd in range(ND):
                        lx = xT[:, dd, i * P : (i + 1) * P]
                        nc.tensor.matmul(
                            pp[:, 0:512], lx, wv[:, dd, 0:512],
                            start=(dd == 0), stop=(dd == ND - 1),
                        )
                        nc.tensor.matmul(
                            pp[:, 512:O], lx, wv[:, dd, 512:O],
                            start=(dd == 0), stop=(dd == ND - 1),
                        )
                    nc.vector.tensor_add(v_sb[:, i, 0:O], pp, bvb)

                # prefetch next batch's activations while attention runs
                if b + 1 < B:
                    xT_next = load_xT(b + 1)

                # ---- S^T + exp ----
                eT = big_pool.tile([P, NQ, N], F16, tag="eT")
                for kk in range(NQ):
                    sp = ps_pool.tile([P, N], F32, tag="ps")
                    for oo in range(NO):
                        lk = kT[:, oo, kk * P : (kk + 1) * P]
                        for h in range(2):
                            nc.tensor.matmul(
                                sp[:, h * 512 : (h + 1) * 512],
                                lk,
                                qT[:, oo, h * 512 : (h + 1) * 512],
                                start=(oo == 0),
                                stop=(oo == NO - 1),
                            )
                    nc.scalar.activation(
                        eT[:, kk, :], sp, Act.Exp, bias=0.0, scale=INV_C
                    )

                # ---- PV + normalize ----
                for i in range(NQ):
                    op_ = ps_pool.tile([P, OA], F32, tag="ps")
                    for kk in range(NQ):
                        le = eT[:, kk, i * P : (i + 1) * P]
                        nc.tensor.matmul(
                            op_[:, 0:512], le, v_sb[:, kk, 0:512],
                            start=(kk == 0), stop=(kk == NQ - 1),
                        )
                        nc.tensor.matmul(
                            op_[:, 512:OA], le, v_sb[:, kk, 512:OA],
                            start=(kk == 0), stop=(kk == NQ - 1),
                        )
                    rs = small_pool.tile([P, 1], F32, tag="rs")
                    nc.vector.reciprocal(rs, op_[:, O : O + 1])
                    on = on_pool.tile([P, O], F32, tag="on")
                    nc.scalar.activation(on, op_[:, 0:O], Act.Copy, bias=0.0, scale=rs)
                    # spread stores across both idle DMA issuers; a single
                    # HWDGE queue drains ~39GB/s and backlogs at the tail
                    e0, e1 = (nc.sync, nc.scalar) if i % 2 == 0 else (nc.scalar, nc.sync)
                    e0.dma_start(out_d[b, i * P : (i + 1) * P, 0:384], on[:, 0:384])
                    e1.dma_start(out_d[b, i * P : (i + 1) * P, 384:O], on[:, 384:O])

    nc.compile()
    return nc


_NC = None


def _get_nc():
    global _NC
    if _NC is None:
        _NC = build()
    return _NC


def run(inputs, trace=False):
    x = np.asarray(inputs["x"], dtype=np.float32)
    wqT = np.ascontiguousarray(np.asarray(inputs["Wq"], dtype=np.float32).T.astype(np.float16))
    wkT = np.ascontiguousarray(np.asarray(inputs["Wk"], dtype=np.float32).T.astype(np.float16))
    wvT = np.ascontiguousarray(np.asarray(inputs["Wv"], dtype=np.float32).T.astype(np.float16))
    bq = np.asarray(inputs["bq"], dtype=np.float32)
    bk = np.asarray(inputs["bk"], dtype=np.float32)
    bv = np.asarray(inputs["bv"], dtype=np.float32)

    xT = np.ascontiguousarray(x.transpose(0, 2, 1).astype(np.float16))  # [32, D, N]

    nc = _get_nc()
    in_maps = []
    for c in range(N_CORES):
        in_maps.append(
            {
                "xT": np.ascontiguousarray(xT[c * B : (c + 1) * B]),
                "wqT": wqT, "wkT": wkT, "wvT": wvT,
                "bq": bq, "bk": bk, "bv": bv,
            }
        )
    res = run_bass_kernel_spmd(
        nc, in_maps, core_ids=list(range(N_CORES)), trace=trace
    )
    out = np.concatenate([res.results[c]["out"] for c in range(N_CORES)], axis=0)
    return out, res


def kernel(**inputs):
    import os

    # tracing needs an NTFF hook that may be absent in the runtime env
    os.environ["BASS_NEVER_TRACE"] = "1"
    out, _ = run(inputs, trace=False)
    return out


# revision 20
# speedup vs baseline: 1.0133x; 1.0060x over previous
"""Attention kernel for Trainium2 (Bass/Tile), 8-core SPMD.

Problem: x[32,1024,768]; Q/K/V = x @ W.T + b (768->768); S = Q K^T / sqrt(768);
P = softmax(S, axis=-1); out = P V.

Sharding: pure data-parallel over batch — 4 batches per core, no collectives.
Host-side prep: x is passed per-batch transposed (xT [d, n]) and the weights
pre-transposed (W^T [d, o]) so every matmul operand already has its
contraction dim on partitions.

Per-core dataflow (per batch), all matmul operands fp16 (full PE rate, FWL
weight loads), fp32 PSUM accumulation:
  - Qt/Kt [o, n]: lhsT = W^T tile, rhs = xT; bias fused into the
    PSUM->SBUF copy on ACT (per-partition bias)
  - V [n, o] natural: lhsT = xT tile, rhs = Wv^T; bias via DVE add with a
    partition-broadcast bias tile; stored fp16 with an appended ones column
  - S^T per k-chunk: lhsT = Kt slice, rhs = Qt; exp(S^T * 1/c) on ACT
    straight out of PSUM, written fp16. No max-subtraction: logits are
    ~N(0,1), |logit| < ~9, so exp() stays finite in fp32/fp16 and the
    result is mathematically identical to the max-subtracted softmax.
  - PV per q-chunk: lhsT = exp(S^T) slice, rhs = V_aug; the ones column
    of V_aug yields the softmax row-sums in the output's last column
  - final PSUM->SBUF copy on ACT applies the 1/rowsum normalization
"""

import math

import numpy as np

import concourse.bass as bass
import concourse.mybir as mybir
import concourse.tile as tile
from concourse import bacc
from concourse.bass_utils import run_bass_kernel_spmd

F32 = mybir.dt.float32
F32R = mybir.dt.float32r
F16 = mybir.dt.float16
BF16 = mybir.dt.bfloat16

N_CORES = 8
B_TOTAL = 32
B = B_TOTAL // N_CORES  # batches per core
N = 1024  # sequence length
D = 768  # embed dim
O = 768  # out dim
P = 128  # partitions
ND = D // P  # 6 d-chunks
NO = O // P  # 6 o-chunks
NQ = N // P  # 8 seq chunks
OA = O + 1  # V width incl. the ones column
SCALE = math.sqrt(float(O)) + 1e-6
INV_C = float(1.0 / SCALE)

Act = mybir.ActivationFunctionType


def build():
    nc = bacc.Bacc("TRN2", target_bir_lowering=False, debug=False)

    xT_d = nc.dram_tensor("xT", [B, D, N], F16, kind="ExternalInput").ap()
    wq_d = nc.dram_tensor("wqT", [D, O], F16, kind="ExternalInput").ap()
    wk_d = nc.dram_tensor("wkT", [D, O], F16, kind="ExternalInput").ap()
    wv_d = nc.dram_tensor("wvT", [D, O], F16, kind="ExternalInput").ap()
    bq_d = nc.dram_tensor("bq", [O], F32, kind="ExternalInput").ap()
    bk_d = nc.dram_tensor("bk", [O], F32, kind="ExternalInput").ap()
    bv_d = nc.dram_tensor("bv", [O], F32, kind="ExternalInput").ap()
    out_d = nc.dram_tensor("out", [B, N, D], F32, kind="ExternalOutput").ap()

    with tile.TileContext(nc) as tc:
        with (
            tc.tile_pool(name="const", bufs=1) as const_pool,
            tc.tile_pool(name="big", bufs=1) as big_pool,
            tc.tile_pool(name="xTp", bufs=2) as xT_pool,
            tc.tile_pool(name="small", bufs=2) as small_pool,
            tc.tile_pool(name="on", bufs=4) as on_pool,
            tc.tile_pool(name="ps", bufs=4, space="PSUM") as ps_pool,
        ):
            def load_xT(b):
                t = xT_pool.tile([P, ND, N], F16, tag="xT", name=f"xT{b}")
                for dd in range(ND):
                    nc.sync.dma_start(t[:, dd, :], xT_d[b, dd * P : (dd + 1) * P, :])
                return t

            # Startup loads: the Sync engine issues DMA descriptors at only
            # ~0.7us each, so spread issue across idle engines and put the
            # first-needed tensors (xT batch 0, wq) on the fastest path.
            w_sb = {nm: const_pool.tile([P, ND, O], F16, tag=f"w{nm}", name=f"w{nm}")
                    for nm in ("q", "k", "v")}
            xT_next = xT_pool.tile([P, ND, N], F16, tag="xT", name="xT0")
            for dd in range(ND):
                nc.sync.dma_start(xT_next[:, dd, :], xT_d[0, dd * P : (dd + 1) * P, :])
                nc.gpsimd.dma_start(w_sb["q"][:, dd, :], wq_d[dd * P : (dd + 1) * P, :])
            for nm, wd in (("k", wk_d), ("v", wv_d)):
                for dd in range(ND):
                    nc.scalar.dma_start(w_sb[nm][:, dd, :], wd[dd * P : (dd + 1) * P, :])
            # per-partition bias layout for Qt/Kt ([o-chunk] -> [128,1] slices)
            bq_sb = const_pool.tile([P, NO], F32, tag="bq")
            nc.sync.dma_start(
                bq_sb, bass.AP(tensor=bq_d.tensor, offset=bq_d.offset, ap=[[1, P], [P, NO]])
            )
            bk_sb = const_pool.tile([P, NO], F32, tag="bk")
            nc.sync.dma_start(
                bk_sb, bass.AP(tensor=bk_d.tensor, offset=bk_d.offset, ap=[[1, P], [P, NO]])
            )
            # bv broadcast across partitions for the V add
            bvb = const_pool.tile([P, O], F32, tag="bvb")
            nc.sync.dma_start(
                bvb, bass.AP(tensor=bv_d.tensor, offset=bv_d.offset, ap=[[0, P], [1, O]])
            )

            for b in range(B):
                xT = xT_next

                # ---- projections ----
                qT = big_pool.tile([P, NO, N], F16, tag="qT")
                kT = big_pool.tile([P, NO, N], F16, tag="kT")
                for nm, dst, bias in (("q", qT, bq_sb), ("k", kT, bk_sb)):
                    w = w_sb[nm]
                    for j in range(NO):
                        pp = ps_pool.tile([P, N], F32, tag="ps")
                        for dd in range(ND):
                            lw = w[:, dd, j * P : (j + 1) * P]
                            for h in range(2):
                                nc.tensor.matmul(
                                    pp[:, h * 512 : (h + 1) * 512],
                                    lw,
                                    xT[:, dd, h * 512 : (h + 1) * 512],
                                    start=(dd == 0),
                                    stop=(dd == ND - 1),
                                )
                        nc.scalar.activation(
                            dst[:, j, :], pp, Act.Identity, bias=bias[:, j : j + 1]
                        )

                v_sb = big_pool.tile([P, NQ, OA], F16, tag="v")
                nc.vector.memset(v_sb[:, :, O : O + 1], 1.0)
                wv = w_sb["v"]
                for i in range(NQ):
                    pp = ps_pool.tile([P, O], F32, tag="ps")
                    for dd in range(ND):
                        lx = xT[:, dd, i * P : (i + 1) * P]
                        nc.tensor.matmul(
                            pp[:, 0:512], lx, wv[:, dd, 0:512],
                            start=(dd == 0), stop=(dd == ND - 1),
                        )
                        nc.tensor.matmul(
                            pp[:, 512:O], lx, wv[:, dd, 512:O],
                            start=(dd == 0), stop=(dd == ND - 1),
                        )
                    nc.vector.tensor_add(v_sb[:, i, 0:O], pp, bvb)

                # prefetch next batch's activations while attention runs
                if b + 1 < B:
                    xT_next = load_xT(b + 1)

                # ---- S^T + exp ----
                eT = big_pool.tile([P, NQ, N], F16, tag="eT")
                for kk in range(NQ):
                    sp = ps_pool.tile([P, N], F32, tag="ps")
                    for oo in range(NO):
                        lk = kT[:, oo, kk * P : (kk + 1) * P]
                        for h in range(2):
                            nc.tensor.matmul(
                                sp[:, h * 512 : (h + 1) * 512],
                                lk,
                                qT[:, oo, h * 512 : (h + 1) * 512],
                                start=(oo == 0),
                                stop=(oo == NO - 1),
                            )
                    nc.scalar.activation(
                        eT[:, kk, :], sp, Act.Exp, bias=0.0, scale=INV_C
                    )

                # ---- PV + normalize ----
                for i in range(NQ):
                    op_ = ps_pool.tile([P, OA], F32, tag="ps")
                    for kk in range(NQ):
                        le = eT[:, kk, i * P : (i + 1) * P]
                        nc.tensor.matmul(
                            op_[:, 0:512], le, v_sb[:, kk, 0:512],
                            start=(kk == 0), stop=(kk == NQ - 1),
                        )
                        nc.tensor.matmul(
                            op_[:, 512:OA], le, v_sb[:, kk, 512:OA],
                            start=(kk == 0), stop=(kk == NQ - 1),
                        )
                    rs = small_pool.tile([P, 1], F32, tag="rs")
                    nc.vector.reciprocal(rs, op_[:, O : O + 1])
                    on = on_pool.tile([P, O], F32, tag="on")
                    nc.scalar.activation(on, op_[:, 0:O], Act.Copy, bias=0.0, scale=rs)
                    # spread stores across both idle DMA issuers; a single
                    # HWDGE queue drains ~39GB/s and backlogs at the tail
                    nc.sync.dma_start(out_d[b, i * P : (i + 1) * P, 0:384], on[:, 0:384])
                    nc.sync.dma_start(out_d[b, i * P : (i + 1) * P, 384:O], on[:, 384:O])

    nc.compile()
    return nc


_NC = None


def _get_nc():
    global _NC
    if _NC is None:
        _NC = build()
    return _NC


def run(inputs, trace=False):
    x = np.asarray(inputs["x"], dtype=np.float32)
    wqT = np.ascontiguousarray(np.asarray(inputs["Wq"], dtype=np.float32).T.astype(np.float16))
    wkT = np.ascontiguousarray(np.asarray(inputs["Wk"], dtype=np.float32).T.astype(np.float16))
    wvT = np.ascontiguousarray(np.asarray(inputs["Wv"], dtype=np.float32).T.astype(np.float16))
    bq = np.asarray(inputs["bq"], dtype=np.float32)
    bk = np.asarray(inputs["bk"], dtype=np.float32)
    bv = np.asarray(inputs["bv"], dtype=np.float32)

    xT = np.ascontiguousarray(x.transpose(0, 2, 1).astype(np.float16))  # [32, D, N]

    nc = _get_nc()
    in_maps = []
    for c in range(N_CORES):
        in_maps.append(
            {
                "xT": np.ascontiguousarray(xT[c * B : (c + 1) * B]),
                "wqT": wqT, "wkT": wkT, "wvT": wvT,
                "bq": bq, "bk": bk, "bv": bv,
            }
        )
    res = run_bass_kernel_spmd(
        nc, in_maps, core_ids=list(range(N_CORES)), trace=trace
    )
    out = np.concatenate([res.results[c]["out"] for c in range(N_CORES)], axis=0)
    return out, res


def kernel(**inputs):
    import os

    # tracing needs an NTFF hook that may be absent in the runtime env
    os.environ["BASS_NEVER_TRACE"] = "1"
    out, _ = run(inputs, trace=False)
    return out
